# revision 21
# baseline (speedup 1.0000x reference)
"""Causal self-attention Trainium2 kernel, v2 (8-core batch x head parallel).

Full inputs in, full output out. Sharding:
  - core c handles batch b = c//4 and 4 heads hs = (c%4)*4 .. hs+4.
  - QKV column-parallel per core: wqkv slice [1024, 6, 128] (q s0|q s1|k s0|
    k s1|v s0|v s1; slab s = head pair), q pre-scaled by 1/sqrt(D).
  - c_proj row-parallel over the core's 256 y-rows; host sums the 4 partials
    per batch (the all-reduce), transposes back, adds bias.

Everything on-device is fp16 except PSUM accumulators (f32 required).
Layouts are transposed end-to-end: xt [C, T], qkT/vT [128(2 heads x 64),
slab, T], V2 tiles [s 128, slab, st, hpair, 65] with a ones column at 64 so
the AV matmul accumulates the softmax denominator Z as psum row 64.

Schedule (single PE instruction stream, engineered for zero PE gaps since
the PE p-state drops to half clock after any idle):
  warm dummy matmuls (DMA landing window)
  -> qkv chunk 0 + V transposes chunk 0
  -> windows j=0..3 (QW=512 query tokens), heads sequential, s-tiles in
     pairs sharing one [128, 1024] f32 psum + one exact-width EXP each;
     per-window preamble computes k/v chunk j, fillers interleave q chunk
     j+1 and c_proj units of chunk j-1 between attention pairs.
Softmax normalize per (head, window): rcp_approx(ys) on DVE -> partition
shift copy of the 1/Z row -> gpsimd partition_broadcast -> one DVE mul into
yT. No PE or ACT involvement, so neither stream stalls.
"""

import math

import numpy as np

import concourse.bass as bass
from concourse import bacc
import concourse.mybir as mybir
from concourse.tile import TileContext
from concourse.bass_utils import run_bass_kernel_spmd

F16 = mybir.dt.float16
F32 = mybir.dt.float32
NPF16 = np.float16

P = 128
B, T, C = 2, 2048, 1024          # T = tokens per batch
H, D = 16, 64
NCORES = 8
HPC = 4                          # heads per core
NCT = C // P                     # 8 contraction tiles
QW = 512                         # query window
NW = T // QW                     # 4 windows
NCH = T // 512                   # 4 token chunks
EXP_BIAS = -4.0
WARM_N = 26                      # dummy matmuls covering the initial DMA wait

EXP = mybir.ActivationFunctionType.Exp


def build_nc(with_bias: bool) -> bacc.Bacc:
    nc = bacc.Bacc(None, target_bir_lowering=False)

    xt = nc.dram_tensor("xt", [C, T], F16, kind="ExternalInput")
    wqkv = nc.dram_tensor("wqkv", [C, 6, P], F16, kind="ExternalInput")
    wp = nc.dram_tensor("wp", [2 * P, C], F16, kind="ExternalInput")
    tri = nc.dram_tensor("tri", [P, P], F16, kind="ExternalInput")
    ident = nc.dram_tensor("ident", [P, P], F16, kind="ExternalInput")
    if with_bias:
        bqkv = nc.dram_tensor("bqkv", [1, 6 * P], F16, kind="ExternalInput")
        ones512 = nc.dram_tensor("ones512", [1, 512], F16, kind="ExternalInput")
    outT = nc.dram_tensor("outT", [C, T], F16, kind="ExternalOutput")

    with TileContext(nc) as tc:
        with (
            tc.tile_pool(name="consts", bufs=1) as consts,
            tc.tile_pool(name="big", bufs=1) as big,
            tc.tile_pool(name="work", bufs=2) as work,
            tc.tile_pool(name="psum", bufs=1, space="PSUM") as ps,
        ):
            # ---- SBUF residents ----
            wqkv_sb = consts.tile([P, NCT, 6, P], F16)
            wp_sb = consts.tile([P, 2, C], F16)
            tri_sb = consts.tile([P, P], F16)
            ident_sb = consts.tile([P, P], F16)
            expb = consts.tile([P, 1], F32)
            nc.vector.memset(expb, EXP_BIAS)
            warm_m = consts.tile([P, 384], F16)
            nc.vector.memset(warm_m, 0.125)
            ones64 = consts.tile([1, 64], F16)
            nc.vector.memset(ones64, 1.0)
            if with_bias:
                bqkv_sb = consts.tile([1, 6 * P], F16)
                ones512_sb = consts.tile([1, 512], F16)

            xt_sb = big.tile([P, NCT, T], F16)
            qkT = big.tile([P, 2, 2, T], F16)        # [row, slab, q|k, tok]
            vT = big.tile([P, 2, T], F16)
            V2 = big.tile([P, 2, 16, 2, 65], F16)    # [s, slab, st, hpair, d+1]
            nc.vector.memset(V2[:, :, :, :, 64:65], 1.0)
            yT = big.tile([P, 2, T], F16)

            # ---- DMA (chunk-0-first order) ----
            for ct in range(NCT):
                nc.sync.dma_start(wqkv_sb[:, ct, :, :],
                                  wqkv[ct * P:(ct + 1) * P, :, :])
                nc.sync.dma_start(xt_sb[:, ct, 0:512], xt[ct * P:(ct + 1) * P, 0:512])
            for c in range(1, NCH):
                for ct in range(NCT):
                    nc.sync.dma_start(xt_sb[:, ct, c * 512:(c + 1) * 512],
                                      xt[ct * P:(ct + 1) * P, c * 512:(c + 1) * 512])
            for s in range(2):
                nc.sync.dma_start(wp_sb[:, s, :], wp[s * P:(s + 1) * P, :])
            nc.sync.dma_start(tri_sb, tri[:, :])
            nc.sync.dma_start(ident_sb, ident[:, :])
            if with_bias:
                nc.sync.dma_start(bqkv_sb, bqkv[:, :])
                nc.sync.dma_start(ones512_sb, ones512[:, :])

            # ---- warm-up: keep the PE busy while chunk-0 DMA lands ----
            warm_ps = ps.tile([P, 512], F32, tag="u1", bufs=2, name="warm")
            for _ in range(WARM_N):
                nc.tensor.matmul(warm_ps[:, 0:384], warm_m[:, 0:P], warm_m,
                                 start=True, stop=True)

            # ---- unit emitters ----
            def qkv_unit(u, c):
                po = ps.tile([P, 512], F32, tag="u1", bufs=2,
                             name=f"qkv_{u}_{c}")
                for ct in range(NCT):
                    nc.tensor.matmul(
                        po, wqkv_sb[:, ct, u, :],
                        xt_sb[:, ct, c * 512:(c + 1) * 512],
                        start=(ct == 0), stop=(ct == NCT - 1 and not with_bias))
                if with_bias:
                    nc.tensor.matmul(po, bqkv_sb[0:1, u * P:(u + 1) * P],
                                     ones512_sb, start=False, stop=True)
                cols = slice(c * 512, (c + 1) * 512)
                if u < 4:                      # q (u 0,1) / k (u 2,3)
                    dst = qkT[:, u % 2, u // 2, cols]
                else:                          # v (u 4,5)
                    dst = vT[:, u - 4, cols]
                if u < 2:
                    # q copies drain on DVE: the ACT queue is deep in EXPs by
                    # the time a window's trailing q units finish, and the u1
                    # psum slot must recycle fast for the next transposes
                    nc.vector.tensor_copy(dst, po)
                else:
                    nc.scalar.copy(dst, po)

            def tp_unit(slab, tb):
                tp = ps.tile([P, 2, 64], F16, tag="u1", bufs=2,
                             name=f"tp_{slab}_{tb}")
                nc.tensor.transpose(tp[:, :, :],
                                    vT[:, slab, tb * P:(tb + 1) * P], ident_sb)
                nc.vector.tensor_copy(V2[:, slab, tb, :, 0:64], tp[:, :, :])

            def proj_unit(of, c):
                po = ps.tile([P, 512], F32, tag="u1", bufs=2,
                             name=f"proj_{of}_{c}")
                for s in range(2):
                    nc.tensor.matmul(po, wp_sb[:, s, of * P:(of + 1) * P],
                                     yT[:, s, c * 512:(c + 1) * 512],
                                     start=(s == 0), stop=(s == 1))
                ot = work.tile([P, 512], F16, tag="ot", bufs=4,
                               name=f"ot_{of}_{c}")
                nc.vector.tensor_copy(ot, po)
                nc.sync.dma_start(outT[of * P:(of + 1) * P,
                                       c * 512:(c + 1) * 512], ot)

            # ---- pre-attention: full chunk 0 ----
            for u in (4, 5, 2, 3, 0, 1):
                qkv_unit(u, 0)
            for slab in range(2):
                for tb in range(4):
                    tp_unit(slab, tb)

            # ---- attention windows ----
            def norm_start(h, j, ys):
                """Emit the ys readers now; return a deferred tail that does
                the PE broadcast of 1/Z and the final normalize mul."""
                slab, hr = h // 2, (h % 2) * 64
                rcpf = work.tile([65, 512], F32, tag="rcpf", bufs=2,
                                 name=f"rcpf_{h}_{j}")
                nc.vector.reciprocal_approx_fast(rcpf, ys)
                ynum = work.tile([64, 512], F16, tag="ynum", bufs=2,
                                 name=f"ynum_{h}_{j}")
                nc.vector.tensor_copy(ynum, ys[0:64, :])
                zst = work.tile([1, 512], F16, tag="zst", bufs=2,
                                name=f"zst_{h}_{j}")
                nc.vector.tensor_copy(zst[0:1, :], rcpf[64:65, :])

                def tail():
                    rb = ps.tile([64, 512], F32, tag="u1", bufs=2,
                                 name=f"rb_{h}_{j}")
                    nc.tensor.matmul(rb, ones64, zst, start=True, stop=True)
                    nc.vector.tensor_mul(
                        yT[hr:hr + 64, slab, j * 512:(j + 1) * 512],
                        ynum, rb)
                return tail

            pending_tails = []
            for j in range(NW):
                qbase = j * QW
                npair = 2 * (j + 1)
                # window preamble: V transposes of chunk j, batched so the
                # PE pays few transpose-mode switches (v/k/q of chunk j were
                # computed as fillers of window j-1, chunk 0 upfront)
                if j >= 1:
                    for slab in range(2):
                        for tb in range(4 * j, 4 * j + 4):
                            tp_unit(slab, tb)
                # fillers: v/k/q of chunk j+1, then proj units per plan
                fillers = []
                if j + 1 < NCH:
                    for u in (4, 5, 2, 3, 0, 1):
                        fillers.append((qkv_unit, u, j + 1))
                if j == 1:
                    for of in range(NCT):
                        fillers.append((proj_unit, of, 0))
                if j == 3:
                    for of in range(NCT):
                        fillers.append((proj_unit, of, 1))
                    for of in range(NCT):
                        fillers.append((proj_unit, of, 2))

                nslots = HPC * npair
                fq, acc = list(fillers), 0.0
                per_slot = len(fq) / nslots

                def pop_fillers():
                    nonlocal acc
                    acc += per_slot
                    while fq and acc >= 1.0:
                        f, *a = fq.pop(0)
                        f(*a)
                        acc -= 1.0

                for h in range(HPC):
                    slab, hr = h // 2, (h % 2) * 64
                    ys = ps.tile([65, 512], F32, tag="ys", bufs=2,
                                 name=f"ys_{h}_{j}")
                    av_fifo = []
                    for p in range(npair):
                        if p == 1 and pending_tails:
                            pending_tails.pop(0)()
                        sp = ps.tile([P, 1024], F32, tag="sp", bufs=2,
                                     name=f"sp_{h}_{j}_{p}")
                        offs, widths, qas = [], [], []
                        off = 0
                        for k in range(2):
                            st = 2 * p + k
                            s0 = st * P
                            qa = max(qbase, s0)
                            w = qbase + QW - qa
                            nc.tensor.matmul(
                                sp[:, off:off + w],
                                qkT[hr:hr + 64, slab, 1, s0:s0 + P],
                                qkT[hr:hr + 64, slab, 0, qa:qa + w],
                                start=True, stop=True)
                            offs.append(off); widths.append(w); qas.append(qa)
                            off += w
                        es = work.tile([P, 1024], F16, tag="es", bufs=6,
                                       name=f"es_{h}_{j}_{p}")
                        tot = offs[1] + widths[1]
                        nc.scalar.activation(es[:, 0:tot], sp[:, 0:tot], EXP,
                                             bias=expb)
                        for k in range(2):
                            if 2 * p + k >= 4 * j:       # diagonal tile
                                nc.gpsimd.tensor_mul(
                                    es[:, offs[k]:offs[k] + P],
                                    es[:, offs[k]:offs[k] + P], tri_sb)

                        def av(p=p, offs=offs, widths=widths, qas=qas, es=es,
                               ys=ys, slab=slab, h=h):
                            for k in range(2):
                                st = 2 * p + k
                                nc.tensor.matmul(
                                    ys[0:65, qas[k] - qbase:
                                       qas[k] - qbase + widths[k]],
                                    V2[:, slab, st, h % 2, :],
                                    es[:, offs[k]:offs[k] + widths[k]],
                                    start=(p == 0 and k == 0),
                                    stop=(p == npair - 1 and k == 1))
                        av_fifo.append(av)
                        if len(av_fifo) > 1:
                            av_fifo.pop(0)()
                        pop_fillers()
                    for f in av_fifo:
                        pop_fillers()
                        f()
                    pending_tails.append(norm_start(h, j, ys))
                while fq:
                    f, *a = fq.pop(0)
                    f(*a)

            # ---- drain: last norm tails + proj of last chunk ----
            for f in pending_tails:
                f()
            pending_tails = []
            for of in range(NCT):
                proj_unit(of, NCH - 1)
    nc.compile()
    return nc


_CACHE = {}


def _get_nc(with_bias: bool) -> bacc.Bacc:
    if with_bias not in _CACHE:
        _CACHE[with_bias] = build_nc(with_bias)
    return _CACHE[with_bias]


def _prep_inputs(x, w_attn, b_attn, w_proj):
    x = np.asarray(x, dtype=np.float32)
    w = np.asarray(w_attn, dtype=np.float32)
    ba = np.asarray(b_attn, dtype=np.float32)
    wpj = np.asarray(w_proj, dtype=np.float32)
    scale = 1.0 / math.sqrt(D)
    with_bias = bool(np.any(ba))

    tri_np = np.triu(np.ones((P, P), dtype=np.float32)).astype(NPF16)
    id_np = np.eye(P, dtype=np.float32).astype(NPF16)
    ones512_np = np.ones((1, 512), dtype=np.float32).astype(NPF16)

    xts = [np.ascontiguousarray(x[b].T).astype(NPF16) for b in range(B)]

    in_maps = []
    for c in range(NCORES):
        b, hg = c // 4, c % 4
        lo = hg * HPC * D                       # 256-wide head slice start
        wq = w[:, lo:lo + 256] * scale
        wk = w[:, C + lo:C + lo + 256]
        wv = w[:, 2 * C + lo:2 * C + lo + 256]
        wqkv_c = np.stack(
            [wq[:, 0:128], wq[:, 128:256], wk[:, 0:128], wk[:, 128:256],
             wv[:, 0:128], wv[:, 128:256]], axis=1).astype(NPF16)
        wp_c = np.ascontiguousarray(wpj[lo:lo + 256, :]).astype(NPF16)
        m = {
            "xt": xts[b],
            "wqkv": wqkv_c,
            "wp": wp_c,
            "tri": tri_np,
            "ident": id_np,
        }
        if with_bias:
            bq = ba[lo:lo + 256] * scale
            bk = ba[C + lo:C + lo + 256]
            bv = ba[2 * C + lo:2 * C + lo + 256]
            m["bqkv"] = np.concatenate(
                [bq[0:128], bq[128:256], bk[0:128], bk[128:256],
                 bv[0:128], bv[128:256]])[None, :].astype(NPF16)
            m["ones512"] = ones512_np
        in_maps.append(m)
    return in_maps, with_bias


def _combine(results, b_proj):
    out = np.empty((B, T, C), dtype=np.float32)
    for b in range(B):
        acc = np.zeros((C, T), dtype=np.float32)
        for c in range(4 * b, 4 * b + 4):
            acc += np.asarray(results[c]["outT"], dtype=np.float32)
        out[b] = acc.T
    out += np.asarray(b_proj, dtype=np.float32)[None, None, :]
    return np.ascontiguousarray(out)


def run(x, w_attn, b_attn, w_proj, b_proj, trace=False, trace_cores=None):
    in_maps, with_bias = _prep_inputs(x, w_attn, b_attn, w_proj)
    nc = _get_nc(with_bias)
    res = run_bass_kernel_spmd(
        nc, in_maps, core_ids=list(range(NCORES)),
        trace=trace, trace_cores=trace_cores,
    )
    return _combine(res.results, b_proj), res


def kernel(x, w_attn, b_attn, w_proj, b_proj):
    out, _ = run(x, w_attn, b_attn, w_proj, b_proj, trace=False)
    return out
